# revision 19
# baseline (speedup 1.0000x reference)
"""GAT layer (gnn_message_passing) on 8 trn2 NeuronCores.

Strategy (dst-sharded, no collectives), v3 "two-pass, grouped windows":

- Each core owns a contiguous 1/8 slice of target nodes. A per-core node
  permutation assigns every node a table row: owned nodes first, then the
  rest; rows 32767 and 50001 are all-zero pad rows. Pass-0 builds a DRAM
  gather table [50002, 256] fp16 where row r = [xp (f-major,h-minor) 128 |
  ones 4 | a_s 4 | zero pad]: xp = x @ W_lin.T, a_s = x @ fold(W_lin, w_s),
  the ones come from an appended 1.0 input row. fp16 rows are 512 B so a
  SWDGE dma_gather descriptor (int16 idx, <=32768 rows per table half)
  fetches ONE node per edge slot - no pair/parity tricks.

- Edges are split by src table row into grid A (row < 32768) and grid B.
  Each grid gets its own pass with its own node arrangement (owned nodes
  sorted by that grid's in-degree, 128-node windows, slot columns =
  per-window max degree -> ~3% padding). Pass A accumulates per-node
  [num | den] partials into DRAM; pass B gathers ALL 6272 partials back
  in one dma_gather and finishes: divide + residual + store.

- Windows are batched into GROUPS of K consecutive windows padded to a
  common column count T_s (K*T_s <= SCAP). Per group: ONE idx DMA, ONE
  SWDGE gather [128, K, T_s, 256], ONE logit chain (a_s copy on ACT, add
  of the precomputed ze+atb, leaky-relu, exp on ACT), ONE msg+denominator
  multiply (fp16 2x: rhs[:,k,t,0:132] = xs * ev over the [33,4] view -
  'ones' cols become the softmax denominator), and ONE fold tree over the
  uniform T_s (K windows fold together). ze (= a_e from slotted
  edge_attr) and the a_t+biases column are computed during pass-0 and
  folded together, so the per-group DVE work is 4 big 2x ops + fold.
"""
import os
import sys
from contextlib import ExitStack

sys.path.insert(0, "/opt/trn_rl_repo")

import numpy as np

N, E = 50000, 1600000
IN_F, EDGE_F, HEADS, OUT_F = 64, 16, 4, 32
NEG_SLOPE = 0.2
NCORES = 8
NODES_PC = N // NCORES            # 6250
NW = (NODES_PC + 127) // 128      # 49 windows/core
WNODES = NW * 128                 # 6272
TROWS = N + 2                     # 50002 table rows (2 pad rows)
TROW = 256                        # fp16 elems per table row (512 B)
SPLIT = 32768
PAD_A = 32767                     # pad row in table half A
PAD_B_LOCAL = N + 1 - SPLIT       # pad row 50001, local idx in half B
ZCHUNK = 512                      # ze precompute mega-chunk columns
SCAP = 64                         # max slot columns per window group
KMAX = 12

# device column j (j < 128) holds logical output column (j%4)*32 + j//4
COLIDX = (np.arange(128) % HEADS) * OUT_F + np.arange(128) // HEADS


def _wrap_idx(idx_slot):
    """[128, ncols] slot idx -> SWDGE wrapped [128, ncols*8] int16."""
    flat = idx_slot.T.reshape(-1)                   # col-major (t, p)
    wrapped = flat.reshape(-1, 16).T                # [16, n/16]
    return np.tile(wrapped, (8, 1)).astype(np.int16)


def _make_groups(Tw):
    """Greedy: consecutive windows, padded to the first (max) T, K*T<=SCAP."""
    groups = []
    w = 0
    g0 = 0
    while w < NW:
        ts = int(Tw[w])
        k = 1
        while (w + k < NW and k < KMAX and (k + 1) * ts <= SCAP):
            k += 1
        groups.append((w, k, ts, g0))
        g0 += k * ts
        w += k
    return groups, g0


def _host_preprocess(x, edge_index, edge_attr, W_lin, w_s, b_s, w_t, b_t,
                     W_edge, w_e, b_e, W_res, bias):
    """Pure index/layout work + weight folding. Returns (common, per_core)."""
    f16 = np.float16
    src = edge_index[0].astype(np.int64)
    dst = edge_index[1].astype(np.int64)

    # ---- weight folding (weights only; standard operator fusion) ----
    wlin_perm = W_lin[COLIDX, :].T                              # [64, 128]
    Dws = (W_lin.reshape(HEADS, OUT_F, IN_F) * w_s[None, :, None]).sum(1).T
    wlin_ext = np.zeros((IN_F + 1, 136), np.float32)
    wlin_ext[:IN_F, 0:128] = wlin_perm
    wlin_ext[IN_F, 128:132] = 1.0
    wlin_ext[:IN_F, 132:136] = Dws

    D = (W_lin.reshape(HEADS, OUT_F, IN_F) * w_t[None, :, None]).sum(1).T
    b_total = float(b_s) + float(b_t) + float(b_e)
    dext = np.vstack([D, np.full((1, HEADS), b_total, np.float32)])

    C = (W_edge.reshape(HEADS, OUT_F, EDGE_F) * w_e[None, :, None]).sum(1)
    crep = np.tile(C.reshape(-1)[None, :], (128, 1))            # [128, 64]

    wrese = np.zeros((IN_F + 1, 128), np.float32)
    wrese[:IN_F] = W_res[COLIDX, :].T
    wrese[IN_F] = bias[COLIDX]

    per_core = []
    TA = np.zeros((NCORES, NW), np.int64)
    TB = np.zeros((NCORES, NW), np.int64)
    pre = []
    for c in range(NCORES):
        lo = c * NODES_PC
        owned = np.arange(lo, lo + NODES_PC)
        emask = (dst >= lo) & (dst < lo + NODES_PC)
        e_ids = np.nonzero(emask)[0]
        dloc = dst[e_ids] - lo                       # 0..6249 (unsorted)

        rest = np.setdiff1d(np.arange(N), owned, assume_unique=True)
        n_lowrest = SPLIT - 1 - NODES_PC
        isB_node = np.zeros(N, bool)
        isB_node[rest[n_lowrest:]] = True
        isB = isB_node[src[e_ids]]

        degA = np.bincount(dloc[~isB], minlength=NODES_PC)
        degB = np.bincount(dloc[isB], minlength=NODES_PC)
        ordA = np.argsort(-degA, kind="stable")      # A-arrangement
        ordB = np.argsort(-degB, kind="stable")      # B-arrangement
        for w in range(NW):
            TA[c, w] = degA[ordA][w * 128:(w + 1) * 128].max()
            TB[c, w] = degB[ordB][w * 128:(w + 1) * 128].max()
        pre.append(dict(owned=owned, e_ids=e_ids, dloc=dloc, isB=isB,
                        ordA=ordA, ordB=ordB, rest=rest, n_lowrest=n_lowrest))

    TAw = np.maximum(TA.max(axis=0), 1)
    TBw = np.maximum(TB.max(axis=0), 1)
    grpA, SUMA = _make_groups(TAw)
    grpB, SUMB = _make_groups(TBw)

    # padded col offset of window w
    def _wcol(groups):
        wc = np.zeros(NW, np.int64)
        for (w0, k, ts, g0) in groups:
            for j in range(k):
                wc[w0 + j] = g0 + j * ts
        return wc
    wcA, wcB = _wcol(grpA), _wcol(grpB)

    for c in range(NCORES):
        p = pre[c]
        owned, e_ids, dloc, isB = p["owned"], p["e_ids"], p["dloc"], p["isB"]
        ordA, ordB = p["ordA"], p["ordB"]
        # table rows: 0..6249 = A-arrangement owned; pad rows 32767, 50001
        perm_pos = np.empty(N, np.int64)
        perm_pos[owned[ordA]] = np.arange(NODES_PC)
        nl = p["n_lowrest"]
        perm_pos[p["rest"][:nl]] = NODES_PC + np.arange(nl)
        perm_pos[p["rest"][nl:]] = SPLIT + np.arange(len(p["rest"]) - nl)

        xT = np.zeros((IN_F + 1, TROWS), np.float32)
        xT[:IN_F, perm_pos] = x.T
        xT[IN_F, perm_pos] = 1.0                     # pad cols stay 0

        xTB = np.zeros((IN_F + 1, WNODES), np.float32)
        xTB[:IN_F, :NODES_PC] = x[owned[ordB]].T
        xTB[IN_F, :NODES_PC] = 1.0

        # partial-gather idx: B-window row (w,pp) -> A-row of same node
        arow_of_node = np.empty(NODES_PC, np.int64)
        arow_of_node[ordA] = np.arange(NODES_PC)
        pg_rows = arow_of_node[ordB]
        pgidx_slot = np.zeros((128, NW), np.int64)
        for w in range(NW):
            n0 = w * 128
            n1 = min(n0 + 128, NODES_PC)
            pgidx_slot[0:n1 - n0, w] = pg_rows[n0:n1]
            if n1 - n0 < 128:                        # fake rows -> row 6250+
                pgidx_slot[n1 - n0:, w] = np.arange(n1 - n0, 128) + n0
        pgidx = _wrap_idx(pgidx_slot)

        per_grid = {}
        for g, (ordG, wcol, SUMT) in (("A", (ordA, wcA, SUMA)),
                                      ("B", (ordB, wcB, SUMB))):
            sel = isB if g == "B" else ~isB
            eg = e_ids[sel]
            pos_of = np.empty(NODES_PC, np.int64)
            pos_of[ordG] = np.arange(NODES_PC)
            dpos = pos_of[dloc[sel]]
            order = np.argsort(dpos, kind="stable")
            eg = eg[order]
            ds = dpos[order]
            starts = np.searchsorted(ds, np.arange(NODES_PC))
            t_of = np.arange(len(ds)) - starts[ds]
            w_of = ds // 128
            p_of = ds % 128
            col = wcol[w_of] + t_of

            s_row = perm_pos[src[eg]]
            if g == "B":
                s_row = s_row - SPLIT
                padrow = PAD_B_LOCAL
            else:
                padrow = PAD_A
            idx_slot = np.full((128, SUMT), padrow, np.int64)
            idx_slot[p_of, col] = s_row
            ea_slot = np.zeros((128, SUMT, EDGE_F), np.float32)
            ea_slot[p_of, col] = edge_attr[eg]

            per_grid[g] = dict(idx=_wrap_idx(idx_slot),
                               ea=ea_slot.reshape(128, SUMT * EDGE_F)
                                         .astype(f16))

        per_core.append(dict(
            xT=xT.astype(f16), xTB=xTB.astype(f16),
            idxA=per_grid["A"]["idx"], idxB=per_grid["B"]["idx"],
            eaA=per_grid["A"]["ea"], eaB=per_grid["B"]["ea"],
            pgidx=pgidx, out_nodes=owned[ordB],
        ))

    common = dict(TAw=TAw, TBw=TBw, grpA=grpA, grpB=grpB,
                  SUMA=SUMA, SUMB=SUMB,
                  wlin_ext=wlin_ext.astype(f16), dext=dext.astype(f16),
                  crep=crep.astype(f16), wrese=wrese.astype(f16))
    return common, per_core


def _build_program(common):
    import concourse.bass as bass
    import concourse.tile as tile
    from concourse import bacc, mybir

    f32 = mybir.dt.float32
    f16 = mybir.dt.float16
    i16 = mybir.dt.int16
    AL = mybir.AluOpType
    grpA, grpB = common["grpA"], common["grpB"]
    SUMA, SUMB = common["SUMA"], common["SUMB"]

    nc = bacc.Bacc("TRN2", target_bir_lowering=False, debug=False,
                   num_devices=NCORES, num_swdge_queues=4)

    xT_d = nc.dram_tensor("xT", [IN_F + 1, TROWS], f16, kind="ExternalInput")
    xTB_d = nc.dram_tensor("xTB", [IN_F + 1, WNODES], f16, kind="ExternalInput")
    idxA_d = nc.dram_tensor("idxA", [128, SUMA * 8], i16, kind="ExternalInput")
    idxB_d = nc.dram_tensor("idxB", [128, SUMB * 8], i16, kind="ExternalInput")
    pgidx_d = nc.dram_tensor("pgidx", [128, NW * 8], i16, kind="ExternalInput")
    eaA_d = nc.dram_tensor("eaA", [128, SUMA * EDGE_F], f16, kind="ExternalInput")
    eaB_d = nc.dram_tensor("eaB", [128, SUMB * EDGE_F], f16, kind="ExternalInput")
    wlin_d = nc.dram_tensor("wlin_ext", [IN_F + 1, 136], f16, kind="ExternalInput")
    dext_d = nc.dram_tensor("dext", [IN_F + 1, HEADS], f16, kind="ExternalInput")
    crep_d = nc.dram_tensor("crep", [128, HEADS * EDGE_F], f16, kind="ExternalInput")
    wrese_d = nc.dram_tensor("wrese", [IN_F + 1, 128], f16, kind="ExternalInput")
    out_d = nc.dram_tensor("out", [WNODES, 128], f16, kind="ExternalOutput")

    with tile.TileContext(nc) as tc, ExitStack() as ctx:
        const = ctx.enter_context(tc.tile_pool(name="const", bufs=1))
        dramp = ctx.enter_context(tc.tile_pool(name="dram", bufs=1, space="DRAM"))
        xp_t = dramp.tile([TROWS, TROW], f16)
        part_t = dramp.tile([WNODES, TROW], f16)

        wlint = const.tile([IN_F + 1, 136], f16)
        nc.sync.dma_start(wlint[:], wlin_d.ap())
        dext_t = const.tile([IN_F + 1, HEADS], f16)
        nc.sync.dma_start(dext_t[:], dext_d.ap())
        crep_t = const.tile([128, HEADS * EDGE_F], f16)
        nc.sync.dma_start(crep_t[:], crep_d.ap())
        wrese_t = const.tile([IN_F + 1, 128], f16)
        nc.sync.dma_start(wrese_t[:], wrese_d.ap())
        xTown = const.tile([IN_F + 1, WNODES], f16)
        nc.sync.dma_start(xTown[:], xT_d.ap()[:, 0:WNODES])
        xTBt = const.tile([IN_F + 1, WNODES], f16)
        nc.sync.dma_start(xTBt[:], xTB_d.ap())
        pgidx_t = const.tile([128, NW * 8], i16)
        nc.sync.dma_start(pgidx_t[:], pgidx_d.ap())
        atbA = const.tile([128, NW * HEADS], f16)
        atbB = const.tile([128, NW * HEADS], f16)
        zeA = const.tile([128, SUMA * HEADS], f16)
        zeB = const.tile([128, SUMB * HEADS], f16)

        # ---- pass-0a: ze (= a_e) for every padded slot of both grids ----
        with tc.tile_pool(name="zep", bufs=2) as zep:
            for ze_t, ea_d, SUMT in ((zeA, eaA_d, SUMA), (zeB, eaB_d, SUMB)):
                c0 = 0
                while c0 < SUMT:
                    cw = min(ZCHUNK, SUMT - c0)
                    eat = zep.tile([128, ZCHUNK * EDGE_F], f16, tag="eat")
                    nc.sync.dma_start(eat[:, :cw * EDGE_F],
                                      ea_d.ap()[:, c0 * EDGE_F:(c0 + cw) * EDGE_F])
                    prode = zep.tile([128, ZCHUNK * HEADS * EDGE_F], f16,
                                     tag="prode")
                    ea_bc = eat[:, :cw * EDGE_F] \
                        .rearrange("p (t k) -> p t k", t=cw) \
                        .rearrange("p t (a k) -> p t a k", a=1) \
                        .broadcast_to([128, cw, HEADS, EDGE_F])
                    crep_bc = crep_t[:].rearrange("p (a f) -> p a f", a=1) \
                        .broadcast_to([128, cw, HEADS * EDGE_F]) \
                        .rearrange("p t (h k) -> p t h k", h=HEADS)
                    pv = prode[:, :cw * HEADS * EDGE_F] \
                        .rearrange("p (g k) -> p g k", k=EDGE_F)
                    nc.vector.tensor_tensor(
                        pv.rearrange("p (t h) k -> p t h k", h=HEADS),
                        ea_bc, crep_bc, op=AL.mult)
                    nc.vector.tensor_tensor(pv[:, :, 0:8], pv[:, :, 0:8],
                                            pv[:, :, 8:16], op=AL.add)
                    nc.vector.tensor_tensor(pv[:, :, 0:4], pv[:, :, 0:4],
                                            pv[:, :, 4:8], op=AL.add)
                    nc.vector.tensor_tensor(pv[:, :, 0:2], pv[:, :, 0:2],
                                            pv[:, :, 2:4], op=AL.add)
                    zv = ze_t[:, c0 * HEADS:(c0 + cw) * HEADS] \
                        .rearrange("p (g a) -> p g a", a=1)
                    nc.vector.tensor_tensor(zv, pv[:, :, 0:1], pv[:, :, 1:2],
                                            op=AL.add)
                    c0 += cw

        # ---- pass-0b: gather table + atb columns ----
        NBLK = (TROWS + 127) // 128
        GB = 8
        SLABW = 12544
        with tc.tile_pool(name="p0slab", bufs=2) as slabp, \
             tc.tile_pool(name="p0", bufs=3) as p0, \
             tc.tile_pool(name="p0ps", bufs=4, space="PSUM") as p0ps:
            nslab = (TROWS + SLABW - 1) // SLABW
            for sl in range(nslab):
                c0 = sl * SLABW
                cw = min(SLABW, TROWS - c0)
                slab = slabp.tile([IN_F + 1, SLABW], f16, tag="slab")
                nc.sync.dma_start(slab[:, :cw], xT_d.ap()[:, c0:c0 + cw])
                b0 = c0 // 128
                bn = (cw + 127) // 128
                for bg in range(b0, b0 + bn, GB):
                    gn = min(GB, b0 + bn - bg)
                    gfull = gn
                    if bg + gn == NBLK and TROWS % 128 != 0:
                        gfull = gn - 1
                    stage = p0.tile([128, GB * 136], f16, tag="stage")
                    for k3 in range(0, gfull, 3):
                        pc = min(3, gfull - k3)
                        ps = p0ps.tile([128, 3 * 136], f32, tag="ps")
                        for j in range(pc):
                            lo = (bg + k3 + j) * 128 - c0
                            nc.tensor.matmul(ps[:, (j * 136):(j + 1) * 136],
                                             slab[:, lo:lo + 128], wlint[:],
                                             start=True, stop=True)
                        if (k3 // 3) % 2 == 0:
                            nc.scalar.copy(
                                stage[:, k3 * 136:(k3 + pc) * 136],
                                ps[:, :pc * 136])
                        else:
                            nc.vector.tensor_copy(
                                stage[:, k3 * 136:(k3 + pc) * 136],
                                ps[:, :pc * 136])
                    if gfull < gn:           # trailing partial row-block
                        b = bg + gfull
                        nb = TROWS - b * 128
                        lo = b * 128 - c0
                        ps = p0ps.tile([128, 3 * 136], f32, tag="ps")
                        nc.tensor.matmul(ps[:nb, 0:136], slab[:, lo:lo + nb],
                                         wlint[:], start=True, stop=True)
                        nc.scalar.copy(stage[:nb, gfull * 136:(gfull + 1) * 136],
                                       ps[:nb, 0:136])
                    if gfull > 0:
                        dst_xp = xp_t[:][128 * bg:128 * (bg + gfull), 0:136] \
                            .rearrange("(k r) f -> r k f", k=gfull)
                        nc.sync.dma_start(
                            dst_xp,
                            stage[:].rearrange("r (k c) -> r k c", c=136)
                                    [:, :gfull, :])
                    if gfull < gn:
                        b = bg + gfull
                        nb = TROWS - b * 128
                        nc.sync.dma_start(
                            xp_t[:][128 * b:128 * b + nb, 0:136],
                            stage[:nb, gfull * 136:(gfull + 1) * 136])
        with tc.tile_pool(name="atbps", bufs=4, space="PSUM") as atbps:
            for w in range(NW):
                ps2 = atbps.tile([128, HEADS], f32, tag="ps2")
                nc.tensor.matmul(ps2[:], xTown[:, w * 128:(w + 1) * 128],
                                 dext_t[:], start=True, stop=True)
                nc.scalar.copy(atbA[:, w * HEADS:(w + 1) * HEADS], ps2[:])
                ps3 = atbps.tile([128, HEADS], f32, tag="ps2")
                nc.tensor.matmul(ps3[:], xTBt[:, w * 128:(w + 1) * 128],
                                 dext_t[:], start=True, stop=True)
                nc.scalar.copy(atbB[:, w * HEADS:(w + 1) * HEADS], ps3[:])

        # fold atb into ze (per window, broadcast over its padded cols)
        for groups, ze_t, atb in ((grpA, zeA, atbA), (grpB, zeB, atbB)):
            for (w0, k, ts, g0) in groups:
                for j in range(k):
                    w = w0 + j
                    cc = g0 + j * ts
                    zv = ze_t[:, cc * HEADS:(cc + ts) * HEADS] \
                        .rearrange("p (t h) -> p t h", t=ts)
                    ab = atb[:, w * HEADS:(w + 1) * HEADS] \
                        .rearrange("p (a h) -> p a h", a=1) \
                        .broadcast_to([128, ts, HEADS])
                    nc.vector.tensor_tensor(zv, zv, ab, op=AL.add)

        # ---- main: pass A then pass B, one GROUP per step ----
        with tc.tile_pool(name="xsp", bufs=2) as xsp, \
             tc.tile_pool(name="idxp", bufs=3) as idxp, \
             tc.tile_pool(name="rhsp", bufs=2) as rhsp, \
             tc.tile_pool(name="sml", bufs=3) as sml, \
             tc.tile_pool(name="pap", bufs=1) as pap, \
             tc.tile_pool(name="outp", bufs=2) as outp, \
             tc.tile_pool(name="mps", bufs=2, space="PSUM") as mps:

            qrr = 0
            paall = None
            for phase in ("A", "B"):
                groups = grpA if phase == "A" else grpB
                idx_d = idxA_d if phase == "A" else idxB_d
                ze_t = zeA if phase == "A" else zeB
                tab = xp_t[:][0:SPLIT, :] if phase == "A" \
                    else xp_t[:][SPLIT:TROWS, :]

                if phase == "B":
                    paall = pap.tile([128, NW, TROW], f16, tag="paall")
                    pw = 0
                    for chunk in (13, 12, 12, 12):
                        nc.gpsimd.dma_gather(
                            paall[:, pw:pw + chunk, :], part_t[:],
                            pgidx_t[:, pw * 8:(pw + chunk) * 8],
                            chunk * 128, chunk * 128, TROW,
                            single_packet=False, queue_num=qrr % 4)
                        qrr += 1
                        pw += chunk

                for (w0, kk, ts, g0) in groups:
                    ncol = kk * ts
                    idxc = idxp.tile([128, SCAP * 8], i16, tag="idxc")
                    nc.sync.dma_start(idxc[:, :ncol * 8],
                                      idx_d.ap()[:, g0 * 8:(g0 + ncol) * 8])
                    xs = xsp.tile([128, SCAP, TROW], f16, tag="xs")
                    # split across all 4 SWDGE queues: one queue drains at
                    # ~1/4 of aggregate DMA bandwidth
                    nsub = 4 if ncol >= 16 else 1
                    base, extra = ncol // nsub, ncol % nsub
                    tpos = 0
                    for si in range(nsub):
                        stn = base + (1 if si < extra else 0)
                        if stn == 0:
                            continue
                        nc.gpsimd.dma_gather(
                            xs[:, tpos:tpos + stn, :], tab,
                            idxc[:, tpos * 8:(tpos + stn) * 8],
                            stn * 128, stn * 128, TROW, single_packet=False,
                            queue_num=qrr % 4)
                        qrr += 1
                        tpos += stn

                    nh = ncol * HEADS
                    # a_s: strided read offloaded to ACT, packed for DVE
                    asp = sml.tile([128, SCAP * HEADS], f16, tag="asp")
                    nc.scalar.copy(
                        asp[:, :nh].rearrange("p (t h) -> p t h", t=ncol),
                        xs[:, 0:ncol, 132:136])
                    u = sml.tile([128, SCAP * HEADS], f16, tag="u")
                    nc.vector.tensor_tensor(
                        u[:, :nh], asp[:, :nh],
                        ze_t[:, g0 * HEADS:(g0 + ncol) * HEADS], op=AL.add)
                    lr = sml.tile([128, SCAP * HEADS], f16, tag="lr")
                    nc.vector.scalar_tensor_tensor(
                        lr[:, :nh], u[:, :nh], NEG_SLOPE, u[:, :nh],
                        op0=AL.mult, op1=AL.max)
                    ev = sml.tile([128, SCAP * HEADS], f16, tag="ev")
                    nc.scalar.activation(ev[:, :nh], lr[:, :nh],
                                         mybir.ActivationFunctionType.Exp)

                    rhs = rhsp.tile([128, SCAP, 132], f16, tag="rhs")
                    ev_bc = ev[:, :nh].rearrange("p (t h) -> p t h", t=ncol) \
                        .rearrange("p t (a h) -> p t a h", a=1) \
                        .broadcast_to([128, ncol, 33, HEADS])
                    xs_v = xs[:, 0:ncol, 0:132] \
                        .rearrange("p t (g h) -> p t g h", h=HEADS)
                    rhs_v = rhs[:, 0:ncol, :] \
                        .rearrange("p t (g h) -> p t g h", h=HEADS)
                    nc.vector.tensor_tensor(rhs_v, xs_v, ev_bc, op=AL.mult)

                    # fold all kk windows together over the uniform ts
                    rq = rhs[:, 0:ncol, :].rearrange("p (k t) f -> p k t f",
                                                     k=kk)
                    n = ts
                    while n > 1:
                        half = n // 2
                        nc.vector.tensor_tensor(
                            rq[:, :, 0:half, :], rq[:, :, 0:half, :],
                            rq[:, :, n - half:n, :], op=AL.add)
                        n -= half

                    if phase == "A":
                        nc.sync.dma_start(
                            part_t[w0 * 128:(w0 + kk) * 128, 0:132]
                            .rearrange("(k r) f -> r k f", k=kk),
                            rq[:, :, 0, :])
                    else:
                        # residuals for the kk windows
                        resg = outp.tile([128, KMAX * 128], f16, tag="resg")
                        for j in range(kk):
                            res_ps = mps.tile([128, 128], f32, tag="res")
                            nc.tensor.matmul(
                                res_ps[:],
                                xTBt[:, (w0 + j) * 128:(w0 + j + 1) * 128],
                                wrese_t[:], start=True, stop=True)
                            nc.scalar.copy(resg[:, j * 128:(j + 1) * 128],
                                           res_ps[:])
                        tot = outp.tile([128, KMAX * 132], f16, tag="tot")
                        totv = tot[:, :kk * 132].rearrange(
                            "p (k f) -> p k f", k=kk)
                        nc.vector.tensor_tensor(
                            totv, rq[:, :, 0, :],
                            paall[:, w0:w0 + kk, 0:132], op=AL.add)
                        dn = outp.tile([128, KMAX * HEADS], f32, tag="dn")
                        nc.scalar.copy(
                            dn[:, :kk * HEADS].rearrange(
                                "p (k h) -> p k h", k=kk),
                            totv[:, :, 128:132])
                        nc.vector.tensor_scalar_max(dn[:, :kk * HEADS],
                                                    dn[:, :kk * HEADS], 1e-4)
                        rec = outp.tile([128, KMAX * HEADS], f32, tag="rec")
                        nc.vector.reciprocal(rec[:, :kk * HEADS],
                                             dn[:, :kk * HEADS])
                        rec16 = outp.tile([128, KMAX * HEADS], f16,
                                          tag="rec16")
                        nc.vector.tensor_copy(rec16[:, :kk * HEADS],
                                              rec[:, :kk * HEADS])
                        out2 = outp.tile([128, KMAX * 128], f16, tag="out2")
                        o2v = out2[:, :kk * 128].rearrange(
                            "p (k g h) -> p k g h", k=kk, h=HEADS)
                        nv = totv[:, :, 0:128].rearrange(
                            "p k (g h) -> p k g h", h=HEADS)
                        rb = rec16[:, :kk * HEADS].rearrange(
                            "p (k a h) -> p k a h", a=1, k=kk) \
                            .broadcast_to([128, kk, 32, HEADS])
                        nc.vector.tensor_tensor(o2v, nv, rb, op=AL.mult)
                        nc.vector.tensor_tensor(out2[:, :kk * 128],
                                                out2[:, :kk * 128],
                                                resg[:, :kk * 128], op=AL.add)
                        nc.sync.dma_start(
                            out_d.ap()[w0 * 128:(w0 + kk) * 128, :]
                            .rearrange("(k r) f -> r k f", k=kk),
                            out2[:, :kk * 128].rearrange(
                                "p (k f) -> p k f", k=kk))

    if not os.environ.get("GAT_SKIP_COMPILE"):
        nc.compile()
    return nc


def kernel(**inputs):
    from concourse.bass_utils import run_bass_kernel_spmd

    args = {k: np.asarray(v) for k, v in inputs.items()}
    common, per_core = _host_preprocess(
        args["x"], args["edge_index"], args["edge_attr"], args["W_lin"],
        args["w_s"], args["b_s"], args["w_t"], args["b_t"], args["W_edge"],
        args["w_e"], args["b_e"], args["W_res"], args["bias"])

    nc = _build_program(common)

    in_maps = []
    for c in range(NCORES):
        pc = per_core[c]
        in_maps.append({
            "xT": pc["xT"], "xTB": pc["xTB"],
            "idxA": pc["idxA"], "idxB": pc["idxB"], "pgidx": pc["pgidx"],
            "eaA": pc["eaA"], "eaB": pc["eaB"],
            "wlin_ext": common["wlin_ext"], "dext": common["dext"],
            "crep": common["crep"], "wrese": common["wrese"],
        })

    res = run_bass_kernel_spmd(nc, in_maps, list(range(NCORES)),
                               trace=bool(os.environ.get("GAT_TRACE")),
                               tmpdir=os.environ.get("GAT_TMPDIR"))
    if os.environ.get("GAT_TRACE"):
        print(f"HW exec time: {res.exec_time_ns} ns")

    out = np.empty((N, HEADS * OUT_F), np.float32)
    for c in range(NCORES):
        dev = res.results[c]["out"][:NODES_PC].astype(np.float32)
        logical = np.empty_like(dev)
        logical[:, COLIDX] = dev                     # device col j -> logical
        out[per_core[c]["out_nodes"]] = logical
    return out


# revision 21
# speedup vs baseline: 1.0884x; 1.0884x over previous
"""GAT layer (gnn_message_passing) on 8 trn2 NeuronCores.

Strategy (dst-sharded, no collectives): two-pass, grouped windows.

- Each core owns a contiguous 1/8 slice of target nodes. A per-core node
  permutation assigns every node a table row: owned nodes first, then the
  rest; rows 32767 and 50001 are all-zero pad rows. Pass-0 builds a DRAM
  gather table [50002, 256] fp16 where row r = [xp (f-major,h-minor) 128 |
  ones 4 | a_s 4 | zero pad]: xp = x @ W_lin.T, a_s = x @ fold(W_lin, w_s),
  the ones come from an appended 1.0 input row. fp16 rows are 512 B so a
  SWDGE dma_gather descriptor (int16 idx, <=32768 rows per table half)
  fetches ONE node per edge slot - no pair/parity tricks.

- Edges are split by src table row into grid A (row < 32768) and grid B.
  Each grid gets its own pass with its own node arrangement (owned nodes
  sorted by that grid's in-degree, 128-node windows, slot columns =
  per-window max degree -> ~3% padding). Pass A accumulates per-node
  [num | den] partials into DRAM; pass B gathers ALL 6272 partials back
  in one dma_gather and finishes: divide + residual + store.

- Windows are batched into GROUPS of K consecutive windows padded to a
  common column count T_s (K*T_s <= SCAP). Per group: ONE idx DMA, ONE
  SWDGE gather [128, K, T_s, 256], ONE logit chain (a_s copy on ACT, add
  of the precomputed ze+atb, leaky-relu, exp on ACT), ONE msg+denominator
  multiply (fp16 2x: rhs[:,k,t,0:132] = xs * ev over the [33,4] view -
  'ones' cols become the softmax denominator), and ONE fold tree over the
  uniform T_s (K windows fold together). ze (= a_e from slotted
  edge_attr) and the a_t+biases column are computed during pass-0 and
  folded together, so the per-group DVE work is 4 big 2x ops + fold.
"""
import os
import sys
from contextlib import ExitStack

sys.path.insert(0, "/opt/trn_rl_repo")

import numpy as np

N, E = 50000, 1600000
IN_F, EDGE_F, HEADS, OUT_F = 64, 16, 4, 32
NEG_SLOPE = 0.2
NCORES = 8
NODES_PC = N // NCORES            # 6250
NW = (NODES_PC + 127) // 128      # 49 windows/core
WNODES = NW * 128                 # 6272
TROWS = N + 2                     # 50002 table rows (2 pad rows)
TROW = 256                        # fp16 elems per table row (512 B)
SPLIT = 32768
PAD_A = 32767                     # pad row in table half A
PAD_B_LOCAL = N + 1 - SPLIT       # pad row 50001, local idx in half B
ZCHUNK = 512                      # ze precompute mega-chunk columns
SCAP = 64                         # max slot columns per window group
KMAX = 12

# device column j (j < 128) holds logical output column (j%4)*32 + j//4
COLIDX = (np.arange(128) % HEADS) * OUT_F + np.arange(128) // HEADS


def _wrap_idx(idx_slot):
    """[128, ncols] slot idx -> SWDGE wrapped [128, ncols*8] int16."""
    flat = idx_slot.T.reshape(-1)                   # col-major (t, p)
    wrapped = flat.reshape(-1, 16).T                # [16, n/16]
    return np.tile(wrapped, (8, 1)).astype(np.int16)


def _make_groups(Tw):
    """Greedy: consecutive windows, padded to the first (max) T, K*T<=SCAP."""
    groups = []
    w = 0
    g0 = 0
    while w < NW:
        ts = int(Tw[w])
        k = 1
        while (w + k < NW and k < KMAX and (k + 1) * ts <= SCAP):
            k += 1
        groups.append((w, k, ts, g0))
        g0 += k * ts
        w += k
    return groups, g0


def _host_preprocess(x, edge_index, edge_attr, W_lin, w_s, b_s, w_t, b_t,
                     W_edge, w_e, b_e, W_res, bias):
    """Pure index/layout work + weight folding. Returns (common, per_core)."""
    f16 = np.float16
    src = edge_index[0].astype(np.int64)
    dst = edge_index[1].astype(np.int64)

    # ---- weight folding (weights only; standard operator fusion) ----
    wlin_perm = W_lin[COLIDX, :].T                              # [64, 128]
    Dws = (W_lin.reshape(HEADS, OUT_F, IN_F) * w_s[None, :, None]).sum(1).T
    wlin_ext = np.zeros((IN_F + 1, 136), np.float32)
    wlin_ext[:IN_F, 0:128] = wlin_perm
    wlin_ext[IN_F, 128:132] = 1.0
    wlin_ext[:IN_F, 132:136] = Dws

    D = (W_lin.reshape(HEADS, OUT_F, IN_F) * w_t[None, :, None]).sum(1).T
    b_total = float(b_s) + float(b_t) + float(b_e)
    dext = np.vstack([D, np.full((1, HEADS), b_total, np.float32)])

    C = (W_edge.reshape(HEADS, OUT_F, EDGE_F) * w_e[None, :, None]).sum(1)
    crep = np.tile(C.reshape(-1)[None, :], (128, 1))            # [128, 64]

    wrese = np.zeros((IN_F + 1, 128), np.float32)
    wrese[:IN_F] = W_res[COLIDX, :].T
    wrese[IN_F] = bias[COLIDX]

    per_core = []
    TA = np.zeros((NCORES, NW), np.int64)
    TB = np.zeros((NCORES, NW), np.int64)
    pre = []
    for c in range(NCORES):
        lo = c * NODES_PC
        owned = np.arange(lo, lo + NODES_PC)
        emask = (dst >= lo) & (dst < lo + NODES_PC)
        e_ids = np.nonzero(emask)[0]
        dloc = dst[e_ids] - lo                       # 0..6249 (unsorted)

        rest = np.setdiff1d(np.arange(N), owned, assume_unique=True)
        n_lowrest = SPLIT - 1 - NODES_PC
        isB_node = np.zeros(N, bool)
        isB_node[rest[n_lowrest:]] = True
        isB = isB_node[src[e_ids]]

        degA = np.bincount(dloc[~isB], minlength=NODES_PC)
        degB = np.bincount(dloc[isB], minlength=NODES_PC)
        ordA = np.argsort(-degA, kind="stable")      # A-arrangement
        ordB = np.argsort(-degB, kind="stable")      # B-arrangement
        for w in range(NW):
            TA[c, w] = degA[ordA][w * 128:(w + 1) * 128].max()
            TB[c, w] = degB[ordB][w * 128:(w + 1) * 128].max()
        pre.append(dict(owned=owned, e_ids=e_ids, dloc=dloc, isB=isB,
                        ordA=ordA, ordB=ordB, rest=rest, n_lowrest=n_lowrest))

    TAw = np.maximum(TA.max(axis=0), 1)
    TBw = np.maximum(TB.max(axis=0), 1)
    grpA, SUMA = _make_groups(TAw)
    grpB, SUMB = _make_groups(TBw)

    # padded col offset of window w
    def _wcol(groups):
        wc = np.zeros(NW, np.int64)
        for (w0, k, ts, g0) in groups:
            for j in range(k):
                wc[w0 + j] = g0 + j * ts
        return wc
    wcA, wcB = _wcol(grpA), _wcol(grpB)

    for c in range(NCORES):
        p = pre[c]
        owned, e_ids, dloc, isB = p["owned"], p["e_ids"], p["dloc"], p["isB"]
        ordA, ordB = p["ordA"], p["ordB"]
        # table rows: 0..6249 = A-arrangement owned; pad rows 32767, 50001
        perm_pos = np.empty(N, np.int64)
        perm_pos[owned[ordA]] = np.arange(NODES_PC)
        nl = p["n_lowrest"]
        perm_pos[p["rest"][:nl]] = NODES_PC + np.arange(nl)
        perm_pos[p["rest"][nl:]] = SPLIT + np.arange(len(p["rest"]) - nl)

        xT = np.zeros((IN_F + 1, TROWS), np.float32)
        xT[:IN_F, perm_pos] = x.T
        xT[IN_F, perm_pos] = 1.0                     # pad cols stay 0

        xTB = np.zeros((IN_F + 1, WNODES), np.float32)
        xTB[:IN_F, :NODES_PC] = x[owned[ordB]].T
        xTB[IN_F, :NODES_PC] = 1.0

        # partial-gather idx: B-window row (w,pp) -> A-row of same node
        arow_of_node = np.empty(NODES_PC, np.int64)
        arow_of_node[ordA] = np.arange(NODES_PC)
        pg_rows = arow_of_node[ordB]
        pgidx_slot = np.zeros((128, NW), np.int64)
        for w in range(NW):
            n0 = w * 128
            n1 = min(n0 + 128, NODES_PC)
            pgidx_slot[0:n1 - n0, w] = pg_rows[n0:n1]
            if n1 - n0 < 128:                        # fake rows -> row 6250+
                pgidx_slot[n1 - n0:, w] = np.arange(n1 - n0, 128) + n0
        pgidx = _wrap_idx(pgidx_slot)

        per_grid = {}
        for g, (ordG, wcol, SUMT) in (("A", (ordA, wcA, SUMA)),
                                      ("B", (ordB, wcB, SUMB))):
            sel = isB if g == "B" else ~isB
            eg = e_ids[sel]
            pos_of = np.empty(NODES_PC, np.int64)
            pos_of[ordG] = np.arange(NODES_PC)
            dpos = pos_of[dloc[sel]]
            order = np.argsort(dpos, kind="stable")
            eg = eg[order]
            ds = dpos[order]
            starts = np.searchsorted(ds, np.arange(NODES_PC))
            t_of = np.arange(len(ds)) - starts[ds]
            w_of = ds // 128
            p_of = ds % 128
            col = wcol[w_of] + t_of

            s_row = perm_pos[src[eg]]
            if g == "B":
                s_row = s_row - SPLIT
                padrow = PAD_B_LOCAL
            else:
                padrow = PAD_A
            idx_slot = np.full((128, SUMT), padrow, np.int64)
            idx_slot[p_of, col] = s_row
            ea_slot = np.zeros((128, SUMT, EDGE_F), np.float32)
            ea_slot[p_of, col] = edge_attr[eg]

            per_grid[g] = dict(idx=_wrap_idx(idx_slot),
                               ea=ea_slot.reshape(128, SUMT * EDGE_F)
                                         .astype(f16))

        per_core.append(dict(
            xT=xT.astype(f16), xTB=xTB.astype(f16),
            idxA=per_grid["A"]["idx"], idxB=per_grid["B"]["idx"],
            eaA=per_grid["A"]["ea"], eaB=per_grid["B"]["ea"],
            pgidx=pgidx, out_nodes=owned[ordB],
        ))

    common = dict(TAw=TAw, TBw=TBw, grpA=grpA, grpB=grpB,
                  SUMA=SUMA, SUMB=SUMB,
                  wlin_ext=wlin_ext.astype(f16), dext=dext.astype(f16),
                  crep=crep.astype(f16), wrese=wrese.astype(f16))
    return common, per_core


def _build_program(common):
    import concourse.bass as bass
    import concourse.tile as tile
    from concourse import bacc, mybir

    f32 = mybir.dt.float32
    f16 = mybir.dt.float16
    i16 = mybir.dt.int16
    AL = mybir.AluOpType
    grpA, grpB = common["grpA"], common["grpB"]
    SUMA, SUMB = common["SUMA"], common["SUMB"]

    nc = bacc.Bacc("TRN2", target_bir_lowering=False, debug=False,
                   num_devices=NCORES, num_swdge_queues=4)

    xT_d = nc.dram_tensor("xT", [IN_F + 1, TROWS], f16, kind="ExternalInput")
    xTB_d = nc.dram_tensor("xTB", [IN_F + 1, WNODES], f16, kind="ExternalInput")
    idxA_d = nc.dram_tensor("idxA", [128, SUMA * 8], i16, kind="ExternalInput")
    idxB_d = nc.dram_tensor("idxB", [128, SUMB * 8], i16, kind="ExternalInput")
    pgidx_d = nc.dram_tensor("pgidx", [128, NW * 8], i16, kind="ExternalInput")
    eaA_d = nc.dram_tensor("eaA", [128, SUMA * EDGE_F], f16, kind="ExternalInput")
    eaB_d = nc.dram_tensor("eaB", [128, SUMB * EDGE_F], f16, kind="ExternalInput")
    wlin_d = nc.dram_tensor("wlin_ext", [IN_F + 1, 136], f16, kind="ExternalInput")
    dext_d = nc.dram_tensor("dext", [IN_F + 1, HEADS], f16, kind="ExternalInput")
    crep_d = nc.dram_tensor("crep", [128, HEADS * EDGE_F], f16, kind="ExternalInput")
    wrese_d = nc.dram_tensor("wrese", [IN_F + 1, 128], f16, kind="ExternalInput")
    out_d = nc.dram_tensor("out", [WNODES, 128], f16, kind="ExternalOutput")

    with tile.TileContext(nc) as tc, ExitStack() as ctx:
        const = ctx.enter_context(tc.tile_pool(name="const", bufs=1))
        dramp = ctx.enter_context(tc.tile_pool(name="dram", bufs=1, space="DRAM"))
        xp_t = dramp.tile([TROWS, TROW], f16)
        part_t = dramp.tile([WNODES, TROW], f16)

        wlint = const.tile([IN_F + 1, 136], f16)
        nc.sync.dma_start(wlint[:], wlin_d.ap())
        dext_t = const.tile([IN_F + 1, HEADS], f16)
        nc.sync.dma_start(dext_t[:], dext_d.ap())
        crep_t = const.tile([128, HEADS * EDGE_F], f16)
        nc.sync.dma_start(crep_t[:], crep_d.ap())
        wrese_t = const.tile([IN_F + 1, 128], f16)
        nc.sync.dma_start(wrese_t[:], wrese_d.ap())
        xTown = const.tile([IN_F + 1, WNODES], f16)
        nc.sync.dma_start(xTown[:], xT_d.ap()[:, 0:WNODES])
        xTBt = const.tile([IN_F + 1, WNODES], f16)
        nc.sync.dma_start(xTBt[:], xTB_d.ap())
        pgidx_t = const.tile([128, NW * 8], i16)
        nc.sync.dma_start(pgidx_t[:], pgidx_d.ap())
        atbA = const.tile([128, NW * HEADS], f16)
        atbB = const.tile([128, NW * HEADS], f16)
        zeA = const.tile([128, SUMA * HEADS], f16)
        zeB = const.tile([128, SUMB * HEADS], f16)

        # ---- pass-0a: ze (= a_e) for every padded slot of both grids ----
        with tc.tile_pool(name="zep", bufs=2) as zep:
            for ze_t, ea_d, SUMT in ((zeA, eaA_d, SUMA), (zeB, eaB_d, SUMB)):
                c0 = 0
                while c0 < SUMT:
                    cw = min(ZCHUNK, SUMT - c0)
                    eat = zep.tile([128, ZCHUNK * EDGE_F], f16, tag="eat")
                    nc.sync.dma_start(eat[:, :cw * EDGE_F],
                                      ea_d.ap()[:, c0 * EDGE_F:(c0 + cw) * EDGE_F])
                    prode = zep.tile([128, ZCHUNK * HEADS * EDGE_F], f16,
                                     tag="prode")
                    ea_bc = eat[:, :cw * EDGE_F] \
                        .rearrange("p (t k) -> p t k", t=cw) \
                        .rearrange("p t (a k) -> p t a k", a=1) \
                        .broadcast_to([128, cw, HEADS, EDGE_F])
                    crep_bc = crep_t[:].rearrange("p (a f) -> p a f", a=1) \
                        .broadcast_to([128, cw, HEADS * EDGE_F]) \
                        .rearrange("p t (h k) -> p t h k", h=HEADS)
                    pv = prode[:, :cw * HEADS * EDGE_F] \
                        .rearrange("p (g k) -> p g k", k=EDGE_F)
                    nc.vector.tensor_tensor(
                        pv.rearrange("p (t h) k -> p t h k", h=HEADS),
                        ea_bc, crep_bc, op=AL.mult)
                    nc.vector.tensor_tensor(pv[:, :, 0:8], pv[:, :, 0:8],
                                            pv[:, :, 8:16], op=AL.add)
                    nc.vector.tensor_tensor(pv[:, :, 0:4], pv[:, :, 0:4],
                                            pv[:, :, 4:8], op=AL.add)
                    nc.vector.tensor_tensor(pv[:, :, 0:2], pv[:, :, 0:2],
                                            pv[:, :, 2:4], op=AL.add)
                    zv = ze_t[:, c0 * HEADS:(c0 + cw) * HEADS] \
                        .rearrange("p (g a) -> p g a", a=1)
                    nc.vector.tensor_tensor(zv, pv[:, :, 0:1], pv[:, :, 1:2],
                                            op=AL.add)
                    c0 += cw

        # ---- pass-0b: gather table + atb columns ----
        NBLK = (TROWS + 127) // 128
        GB = 8
        SLABW = 12544
        with tc.tile_pool(name="p0slab", bufs=2) as slabp, \
             tc.tile_pool(name="p0", bufs=4) as p0, \
             tc.tile_pool(name="p0ps", bufs=6, space="PSUM") as p0ps:
            nslab = (TROWS + SLABW - 1) // SLABW
            for sl in range(nslab):
                c0 = sl * SLABW
                cw = min(SLABW, TROWS - c0)
                slab = slabp.tile([IN_F + 1, SLABW], f16, tag="slab")
                nc.sync.dma_start(slab[:, :cw], xT_d.ap()[:, c0:c0 + cw])
                b0 = c0 // 128
                bn = (cw + 127) // 128
                for bg in range(b0, b0 + bn, GB):
                    gn = min(GB, b0 + bn - bg)
                    gfull = gn
                    if bg + gn == NBLK and TROWS % 128 != 0:
                        gfull = gn - 1
                    stage = p0.tile([128, GB * 136], f16, tag="stage")
                    for k3 in range(0, gfull, 3):
                        pc = min(3, gfull - k3)
                        ps = p0ps.tile([128, 3 * 136], f32, tag="ps")
                        for j in range(pc):
                            lo = (bg + k3 + j) * 128 - c0
                            nc.tensor.matmul(ps[:, (j * 136):(j + 1) * 136],
                                             slab[:, lo:lo + 128], wlint[:],
                                             start=True, stop=True)
                        if (k3 // 3) % 2 == 0:
                            nc.scalar.copy(
                                stage[:, k3 * 136:(k3 + pc) * 136],
                                ps[:, :pc * 136])
                        else:
                            nc.vector.tensor_copy(
                                stage[:, k3 * 136:(k3 + pc) * 136],
                                ps[:, :pc * 136])
                    if gfull < gn:           # trailing partial row-block
                        b = bg + gfull
                        nb = TROWS - b * 128
                        lo = b * 128 - c0
                        ps = p0ps.tile([128, 3 * 136], f32, tag="ps")
                        nc.tensor.matmul(ps[:nb, 0:136], slab[:, lo:lo + nb],
                                         wlint[:], start=True, stop=True)
                        nc.scalar.copy(stage[:nb, gfull * 136:(gfull + 1) * 136],
                                       ps[:nb, 0:136])
                    if gfull > 0:
                        dst_xp = xp_t[:][128 * bg:128 * (bg + gfull), 0:136] \
                            .rearrange("(k r) f -> r k f", k=gfull)
                        nc.sync.dma_start(
                            dst_xp,
                            stage[:].rearrange("r (k c) -> r k c", c=136)
                                    [:, :gfull, :])
                    if gfull < gn:
                        b = bg + gfull
                        nb = TROWS - b * 128
                        nc.sync.dma_start(
                            xp_t[:][128 * b:128 * b + nb, 0:136],
                            stage[:nb, gfull * 136:(gfull + 1) * 136])
        with tc.tile_pool(name="atbps", bufs=4, space="PSUM") as atbps:
            for w in range(NW):
                ps2 = atbps.tile([128, HEADS], f32, tag="ps2")
                nc.tensor.matmul(ps2[:], xTown[:, w * 128:(w + 1) * 128],
                                 dext_t[:], start=True, stop=True)
                nc.scalar.copy(atbA[:, w * HEADS:(w + 1) * HEADS], ps2[:])
                ps3 = atbps.tile([128, HEADS], f32, tag="ps2")
                nc.tensor.matmul(ps3[:], xTBt[:, w * 128:(w + 1) * 128],
                                 dext_t[:], start=True, stop=True)
                nc.scalar.copy(atbB[:, w * HEADS:(w + 1) * HEADS], ps3[:])

        # fold atb into ze (per window, broadcast over its padded cols)
        for groups, ze_t, atb in ((grpA, zeA, atbA), (grpB, zeB, atbB)):
            for (w0, k, ts, g0) in groups:
                for j in range(k):
                    w = w0 + j
                    cc = g0 + j * ts
                    zv = ze_t[:, cc * HEADS:(cc + ts) * HEADS] \
                        .rearrange("p (t h) -> p t h", t=ts)
                    ab = atb[:, w * HEADS:(w + 1) * HEADS] \
                        .rearrange("p (a h) -> p a h", a=1) \
                        .broadcast_to([128, ts, HEADS])
                    nc.vector.tensor_tensor(zv, zv, ab, op=AL.add)

        # ---- main: pass A then pass B, one GROUP per step ----
        with tc.tile_pool(name="xsp", bufs=2) as xsp, \
             tc.tile_pool(name="idxp", bufs=4) as idxp, \
             tc.tile_pool(name="rhsp", bufs=2) as rhsp, \
             tc.tile_pool(name="sml", bufs=4) as sml, \
             tc.tile_pool(name="pap", bufs=1) as pap, \
             tc.tile_pool(name="outp", bufs=2) as outp, \
             tc.tile_pool(name="mps", bufs=2, space="PSUM") as mps:

            qrr = 0
            paall = None
            for phase in ("A", "B"):
                groups = grpA if phase == "A" else grpB
                idx_d = idxA_d if phase == "A" else idxB_d
                ze_t = zeA if phase == "A" else zeB
                tab = xp_t[:][0:SPLIT, :] if phase == "A" \
                    else xp_t[:][SPLIT:TROWS, :]

                if phase == "B":
                    paall = pap.tile([128, NW, TROW], f16, tag="paall")
                    pw = 0
                    for chunk in (13, 12, 12, 12):
                        nc.gpsimd.dma_gather(
                            paall[:, pw:pw + chunk, :], part_t[:],
                            pgidx_t[:, pw * 8:(pw + chunk) * 8],
                            chunk * 128, chunk * 128, TROW,
                            single_packet=False, queue_num=qrr % 4)
                        qrr += 1
                        pw += chunk

                for (w0, kk, ts, g0) in groups:
                    ncol = kk * ts
                    idxc = idxp.tile([128, SCAP * 8], i16, tag="idxc")
                    nc.sync.dma_start(idxc[:, :ncol * 8],
                                      idx_d.ap()[:, g0 * 8:(g0 + ncol) * 8])
                    xs = xsp.tile([128, SCAP, TROW], f16, tag="xs")
                    # split across all 4 SWDGE queues: one queue drains at
                    # ~1/4 of aggregate DMA bandwidth
                    nsub = 4 if ncol >= 16 else 1
                    base, extra = ncol // nsub, ncol % nsub
                    tpos = 0
                    for si in range(nsub):
                        stn = base + (1 if si < extra else 0)
                        if stn == 0:
                            continue
                        nc.gpsimd.dma_gather(
                            xs[:, tpos:tpos + stn, :], tab,
                            idxc[:, tpos * 8:(tpos + stn) * 8],
                            stn * 128, stn * 128, TROW, single_packet=False,
                            queue_num=qrr % 4)
                        qrr += 1
                        tpos += stn

                    nh = ncol * HEADS
                    # a_s: strided read offloaded to ACT, packed for DVE
                    asp = sml.tile([128, SCAP * HEADS], f16, tag="asp")
                    nc.scalar.copy(
                        asp[:, :nh].rearrange("p (t h) -> p t h", t=ncol),
                        xs[:, 0:ncol, 132:136])
                    u = sml.tile([128, SCAP * HEADS], f16, tag="u")
                    nc.vector.tensor_tensor(
                        u[:, :nh], asp[:, :nh],
                        ze_t[:, g0 * HEADS:(g0 + ncol) * HEADS], op=AL.add)
                    lr = sml.tile([128, SCAP * HEADS], f16, tag="lr")
                    nc.vector.scalar_tensor_tensor(
                        lr[:, :nh], u[:, :nh], NEG_SLOPE, u[:, :nh],
                        op0=AL.mult, op1=AL.max)
                    ev = sml.tile([128, SCAP * HEADS], f16, tag="ev")
                    nc.scalar.activation(ev[:, :nh], lr[:, :nh],
                                         mybir.ActivationFunctionType.Exp)

                    rhs = rhsp.tile([128, SCAP, 132], f16, tag="rhs")
                    ev_bc = ev[:, :nh].rearrange("p (t h) -> p t h", t=ncol) \
                        .rearrange("p t (a h) -> p t a h", a=1) \
                        .broadcast_to([128, ncol, 33, HEADS])
                    xs_v = xs[:, 0:ncol, 0:132] \
                        .rearrange("p t (g h) -> p t g h", h=HEADS)
                    rhs_v = rhs[:, 0:ncol, :] \
                        .rearrange("p t (g h) -> p t g h", h=HEADS)
                    nc.vector.tensor_tensor(rhs_v, xs_v, ev_bc, op=AL.mult)

                    # fold all kk windows together over the uniform ts
                    rq = rhs[:, 0:ncol, :].rearrange("p (k t) f -> p k t f",
                                                     k=kk)
                    n = ts
                    while n > 1:
                        half = n // 2
                        nc.vector.tensor_tensor(
                            rq[:, :, 0:half, :], rq[:, :, 0:half, :],
                            rq[:, :, n - half:n, :], op=AL.add)
                        n -= half

                    if phase == "A":
                        nc.sync.dma_start(
                            part_t[w0 * 128:(w0 + kk) * 128, 0:132]
                            .rearrange("(k r) f -> r k f", k=kk),
                            rq[:, :, 0, :])
                    else:
                        # residuals for the kk windows
                        resg = outp.tile([128, KMAX * 128], f16, tag="resg")
                        for j in range(kk):
                            res_ps = mps.tile([128, 128], f32, tag="res")
                            nc.tensor.matmul(
                                res_ps[:],
                                xTBt[:, (w0 + j) * 128:(w0 + j + 1) * 128],
                                wrese_t[:], start=True, stop=True)
                            nc.scalar.copy(resg[:, j * 128:(j + 1) * 128],
                                           res_ps[:])
                        tot = outp.tile([128, KMAX * 132], f16, tag="tot")
                        totv = tot[:, :kk * 132].rearrange(
                            "p (k f) -> p k f", k=kk)
                        nc.vector.tensor_tensor(
                            totv, rq[:, :, 0, :],
                            paall[:, w0:w0 + kk, 0:132], op=AL.add)
                        dn = outp.tile([128, KMAX * HEADS], f32, tag="dn")
                        nc.scalar.copy(
                            dn[:, :kk * HEADS].rearrange(
                                "p (k h) -> p k h", k=kk),
                            totv[:, :, 128:132])
                        nc.vector.tensor_scalar_max(dn[:, :kk * HEADS],
                                                    dn[:, :kk * HEADS], 1e-4)
                        rec = outp.tile([128, KMAX * HEADS], f32, tag="rec")
                        nc.vector.reciprocal(rec[:, :kk * HEADS],
                                             dn[:, :kk * HEADS])
                        rec16 = outp.tile([128, KMAX * HEADS], f16,
                                          tag="rec16")
                        nc.vector.tensor_copy(rec16[:, :kk * HEADS],
                                              rec[:, :kk * HEADS])
                        out2 = outp.tile([128, KMAX * 128], f16, tag="out2")
                        o2v = out2[:, :kk * 128].rearrange(
                            "p (k g h) -> p k g h", k=kk, h=HEADS)
                        nv = totv[:, :, 0:128].rearrange(
                            "p k (g h) -> p k g h", h=HEADS)
                        rb = rec16[:, :kk * HEADS].rearrange(
                            "p (k a h) -> p k a h", a=1, k=kk) \
                            .broadcast_to([128, kk, 32, HEADS])
                        nc.vector.tensor_tensor(o2v, nv, rb, op=AL.mult)
                        nc.vector.tensor_tensor(out2[:, :kk * 128],
                                                out2[:, :kk * 128],
                                                resg[:, :kk * 128], op=AL.add)
                        nc.sync.dma_start(
                            out_d.ap()[w0 * 128:(w0 + kk) * 128, :]
                            .rearrange("(k r) f -> r k f", k=kk),
                            out2[:, :kk * 128].rearrange(
                                "p (k f) -> p k f", k=kk))

    if not os.environ.get("GAT_SKIP_COMPILE"):
        nc.compile()
    return nc


def kernel(**inputs):
    from concourse.bass_utils import run_bass_kernel_spmd

    args = {k: np.asarray(v) for k, v in inputs.items()}
    common, per_core = _host_preprocess(
        args["x"], args["edge_index"], args["edge_attr"], args["W_lin"],
        args["w_s"], args["b_s"], args["w_t"], args["b_t"], args["W_edge"],
        args["w_e"], args["b_e"], args["W_res"], args["bias"])

    nc = _build_program(common)

    in_maps = []
    for c in range(NCORES):
        pc = per_core[c]
        in_maps.append({
            "xT": pc["xT"], "xTB": pc["xTB"],
            "idxA": pc["idxA"], "idxB": pc["idxB"], "pgidx": pc["pgidx"],
            "eaA": pc["eaA"], "eaB": pc["eaB"],
            "wlin_ext": common["wlin_ext"], "dext": common["dext"],
            "crep": common["crep"], "wrese": common["wrese"],
        })

    res = run_bass_kernel_spmd(nc, in_maps, list(range(NCORES)),
                               trace=bool(os.environ.get("GAT_TRACE")),
                               tmpdir=os.environ.get("GAT_TMPDIR"))
    if os.environ.get("GAT_TRACE"):
        print(f"HW exec time: {res.exec_time_ns} ns")

    out = np.empty((N, HEADS * OUT_F), np.float32)
    for c in range(NCORES):
        dev = res.results[c]["out"][:NODES_PC].astype(np.float32)
        logical = np.empty_like(dev)
        logical[:, COLIDX] = dev                     # device col j -> logical
        out[per_core[c]["out_nodes"]] = logical
    return out
